# revision 14
# baseline (speedup 1.0000x reference)
"""CARAFE exact-fp32 hybrid kernel.

Natural layout (channels on partitions). Per chunk of 1024 output pixels
(2 source rows x 4 output rows... 2 row-pairs), per tap:
  - PE: 6 selection-matmuls broadcast mask row t to all 128 partitions.
    Masks are split hi/mid/lo into three bf16 arrays (host-side); the three
    K=25 bf16 matmuls accumulate in fp32 PSUM, reconstructing the fp32 mask
    to ~2^-24 -- effectively exact.  out = sel_t.T @ mask_s
  - DVE: fp32 tensor_tensor multiply feat_window x mb -> tmp (or directly
    into an accumulator for the two chain-head taps).
  - adds: two independent accumulator chains so DVE and GPSIMD never wait on
    each other: acc_d (DVE chain) and acc_g (GPSIMD chain), combined at the
    end with one DVE add.  All adds are fp32.
Everything in the value path is fp32 (or exactly representable) -> ~1e-7.
"""

import numpy as np

N, C, H, W = 2, 128, 128, 128
K, S, R = 5, 2, 2
NT = K * K
HQ = 4
HPC = H // HQ  # 32 source rows per core
PROWS, PCOLS = HPC + 2 * R, W + 2 * R  # 36, 132
OROWS = 2 * HPC  # 64 output rows per core
NCORES = 8
NSPLIT = 3  # bf16 mask splits
GPS_TAPS = 19  # taps 1..GPS_TAPS accumulate on the second chain (tap 1 = head)
# GPSIMD adds measured 8x slower than DVE on HW (dispatch/join overhead) --
# both chains run on the DVE; two chains still help instruction independence.
USE_GPS = False

_prog_cache = {}


def _build_program(repeats=1):
    import concourse.bacc as bacc
    import concourse.mybir as mybir
    from concourse.tile import TileContext

    f32 = mybir.dt.float32
    bf16 = mybir.dt.bfloat16

    nc = bacc.Bacc(None, target_bir_lowering=False)
    fp = nc.dram_tensor("featp", [C, PROWS * PCOLS], f32, kind="ExternalInput")
    # three bf16 mask splits concatenated along the free dim (all operands
    # base-partition 0: accumulation groups with mixed base partitions fault)
    mk = nc.dram_tensor(
        "maskS", [NT, NSPLIT * OROWS * 2 * W], bf16, kind="ExternalInput"
    )
    sel = nc.dram_tensor("sel", [NT, NT * 128], bf16, kind="ExternalInput")
    out = nc.dram_tensor("out", [C, OROWS * 2 * W], f32, kind="ExternalOutput")

    with TileContext(nc) as tc:
        with (
            tc.tile_pool(name="const", bufs=1) as cpool,
            tc.tile_pool(name="feat", bufs=1) as fpool,
            tc.tile_pool(name="mask", bufs=1) as mpool,
            tc.tile_pool(name="tmp", bufs=8) as tpool,
            tc.tile_pool(name="accs", bufs=3) as apool,
            tc.tile_pool(name="stage", bufs=3) as spool,
            tc.tile_pool(name="mb", bufs=3, space="PSUM") as mbpool,
        ):
            sel_sb = cpool.tile([NT, NT * 128], bf16)
            nc.sync.dma_start(out=sel_sb[:], in_=sel[:])
            feat_sb = fpool.tile([C, PROWS * PCOLS], f32)
            nc.sync.dma_start(out=feat_sb[:], in_=fp[:])
            mask_sb = mpool.tile([NT, NSPLIT * OROWS * 2 * W], bf16)
            nc.sync.dma_start(out=mask_sb[:], in_=mk[:])

            featv = feat_sb[:].rearrange("c (r w) -> c r w", w=PCOLS)
            # per split s: [25, s, blk, w, sh, sw]
            maskv = mask_sb[:].rearrange(
                "t (s blk sh w sw) -> t s blk w sh sw", s=NSPLIT, sh=2, w=W, sw=2
            )
            outv = out[:].rearrange("c (oh ow) -> c oh ow", ow=2 * W)

            import contextlib

            rep_ctx = tc.For_i(0, repeats, 1) if repeats > 1 else contextlib.nullcontext()
            with rep_ctx:
                _chunks(nc, tc, featv, maskv, outv, sel_sb, tpool, apool, spool, mbpool)
    nc.finalize()
    return nc


def _chunks(nc, tc, featv, maskv, outv, sel_sb, tpool, apool, spool, mbpool):
    import concourse.mybir as mybir

    f32 = mybir.dt.float32

    # tap 0 heads the DVE chain; tap 1 heads the GPSIMD chain; taps 2..GPS_TAPS
    # add on GPSIMD (early, so the GPS chain drains tmps as DVE produces them);
    # taps GPS_TAPS+1..24 add on DVE.
    g0 = 1  # head of gpsimd chain
    nchunks = HPC // 2
    for chunk in range(nchunks):
        hl = 2 * chunk
        acc_d = apool.tile([128, 1024], f32, tag="acc_d")
        acc_g = apool.tile([128, 1024], f32, tag="acc_g")
        for t in range(NT):
            i, j = divmod(t, K)
            mb = mbpool.tile([128, 1024], f32)
            lhsT_sel = sel_sb[:, 128 * t : 128 * (t + 1)]
            for hh in range(2):
                for s in range(NSPLIT):
                    rhs = maskv[:, s, 2 * chunk + hh]
                    nc.tensor.matmul(
                        mb[:, 512 * hh : 512 * (hh + 1)],
                        lhsT=lhsT_sel,
                        rhs=rhs,
                        start=(s == 0),
                        stop=(s == NSPLIT - 1),
                    )
            fap = featv[:, hl + i : hl + i + 2, j : j + W]
            fap = fap[:, :, :, None].to_broadcast([C, 2, W, 4])
            if t == 0:
                dst = acc_d
            elif t == g0:
                dst = acc_g
            else:
                dst = tpool.tile([128, 1024], f32, tag="tmp")
            nc.vector.tensor_tensor(dst[:], fap, mb[:], mybir.AluOpType.mult)
            if t != 0 and t != g0:
                if t <= GPS_TAPS:
                    (nc.gpsimd if USE_GPS else nc.vector).tensor_tensor(
                        acc_g[:], acc_g[:], dst[:], mybir.AluOpType.add
                    )
                else:
                    nc.vector.tensor_tensor(
                        acc_d[:], acc_d[:], dst[:], mybir.AluOpType.add
                    )
        # combine chains on DVE; ACT reorders (hh,w,sh,sw)->(oh,ow) into the
        # stage tile; contiguous DMA out
        nc.vector.tensor_tensor(acc_d[:], acc_d[:], acc_g[:], mybir.AluOpType.add)
        stage = spool.tile([128, 1024], f32)
        av = acc_d[:].rearrange("c (hh w sh sw) -> c hh sh w sw", hh=2, w=W, sh=2, sw=2)
        for hh in range(2):
            nc.scalar.copy(stage[:, 512 * hh : 512 * (hh + 1)], av[:, hh])
        nc.sync.dma_start(
            out=outv[:, 4 * chunk : 4 * chunk + 4, :], in_=stage[:]
        )


def get_program(repeats=1):
    key = ("nc", repeats)
    if key not in _prog_cache:
        _prog_cache[key] = _build_program(repeats)
    return _prog_cache[key]


def make_in_maps(features, masks):
    features = np.asarray(features, dtype=np.float32)
    masks = np.asarray(masks, dtype=np.float32)

    def bf16(x):
        # round-to-nearest-even fp32 -> bf16, returned as fp32 values
        u = x.view(np.uint32)
        r = ((u >> 16) + ((u >> 15) & 1)).astype(np.uint32) << 16
        return r.view(np.float32)

    sel = np.zeros((NT, NT * 128), dtype=np.float32)
    for t in range(NT):
        sel[t, 128 * t : 128 * (t + 1)] = 1.0
    sel_b = _to_bf16_bytes(sel)

    in_maps = []
    for core in range(NCORES):
        n, q = divmod(core, HQ)
        h0 = HPC * q
        featp = np.zeros((C, PROWS, PCOLS), np.float32)
        lo = max(h0 - R, 0)
        hi = min(h0 + HPC + R, H)
        featp[:, (lo - (h0 - R)) : (hi - (h0 - R)), R : R + W] = features[n, :, lo:hi, :]
        m = masks[n, :, 2 * h0 : 2 * h0 + OROWS, :].reshape(NT, -1)
        m_hi = bf16(m)
        m_mid = bf16(m - m_hi)
        m_lo = bf16(m - m_hi - m_mid)
        maskS = np.concatenate([m_hi, m_mid, m_lo], axis=1)  # [25, 3*16384]
        in_maps.append(
            {
                "featp": featp.reshape(C, -1),
                "maskS": _to_bf16_bytes(maskS),
                "sel": sel_b,
            }
        )
    return in_maps


def _to_bf16_bytes(x32):
    """fp32 array whose values are bf16-representable -> ml_dtypes/np bf16 view."""
    import ml_dtypes

    return x32.astype(ml_dtypes.bfloat16)


def gather_output(results):
    out = np.empty((N, C, 2 * H, 2 * W), np.float32)
    for core in range(NCORES):
        n, q = divmod(core, HQ)
        oh0 = 2 * HPC * q
        out[n, :, oh0 : oh0 + OROWS, :] = results[core]["out"].reshape(C, OROWS, 2 * W)
    return out


def kernel(features, masks):
    from concourse.bass_utils import run_bass_kernel_spmd

    nc = get_program()
    in_maps = make_in_maps(features, masks)
    res = run_bass_kernel_spmd(nc, in_maps, core_ids=list(range(NCORES)))
    return gather_output(res.results)
